# revision 7
# baseline (speedup 1.0000x reference)
"""Single-head causal attention on 8 Trainium2 NeuronCores (Bass/Tile), v5.

v4 -> v5: key-split sharding. Each core pair splits the KEYS of its
batch item (core A takes 256-key blocks {0,3,4,7}, core B {1,2,5,6});
both cores process ALL 2048 queries and ship unnormalized partial
O~ = sum_own w v~ (f32) plus l = sum_own w; the host combines
(O~_A + O~_B) / (l_A + l_B). This dedupes the K/V projections across
the pair (the dominant PE cost in v3/v4) at the price of duplicating
the Q projection and ~7us of causal padding.

SPMD uniformity: both parities run the identical instruction stream --
step i processes one own key block at padded query width 2048-512*i;
the first 512 columns of each step are masked (zero/triangle/full
patterns shipped per core), which absorbs the parity difference.

Projections stay 3-term scaled-fp8 DoubleRow (see v4). Attention
tensors (qT, kT, v, w, masks) are bf16; O accumulates f32 in SBUF.
"""

import sys

import numpy as np

for _p in ("/opt/trn_rl_repo", "/root/.axon_site/_ro/trn_rl_repo"):
    if _p not in sys.path:
        sys.path.append(_p)

B, S, D = 4, 2048, 1024
P = 128
NS = 24  # fp8 variant slices per contraction (3 per 128-deep k-tile)
NSTEP = 4  # own key blocks per core (256 keys each)
OWN = {0: (0, 3, 4, 7), 1: (1, 2, 5, 6)}  # parity -> global key blocks
SCALE = 1.0 / np.sqrt(np.float32(D))

_cached = {}


def _build_bass():
    import concourse.bacc as bacc
    import concourse.mybir as mybir
    import concourse.tile as tile
    from contextlib import ExitStack

    f32 = mybir.dt.float32
    bf16 = mybir.dt.bfloat16
    fp8 = mybir.dt.float8e4
    DR = mybir.MatmulPerfMode.DoubleRow

    nc = bacc.Bacc("TRN2")
    # Host layouts (partition-major, contiguous rows):
    #   xT:    [128, i*(24*256) + s*256 + c]   own key blocks' x^T fp8 slices
    #   xqT:   [128, qh*(24*1024) + s*1024 + c] all queries' x^T fp8 slices
    #   W*:    [128, et*(24*128) + s*128 + e]   fp8 slices, e-tile-major
    #   masks: [128, (2*i+kt)*512 + c]          step i's first-4-slot masks
    xt_d = nc.declare_dram_parameter("xT", [P, NSTEP * NS * 256], fp8, isOutput=False)
    xq_d = nc.declare_dram_parameter("xqT", [P, 2 * NS * 1024], fp8, isOutput=False)
    wq_d = nc.declare_dram_parameter("Wq", [P, NS * D], fp8, isOutput=False)
    wk_d = nc.declare_dram_parameter("Wk", [P, NS * D], fp8, isOutput=False)
    wv_d = nc.declare_dram_parameter("Wv", [P, NS * D], fp8, isOutput=False)
    masks_d = nc.declare_dram_parameter("masks", [P, 8 * 512], bf16, isOutput=False)
    out_d = nc.declare_dram_parameter("out", [S, D], f32, isOutput=True)
    l_d = nc.declare_dram_parameter("l", [P, 16], f32, isOutput=True)

    ET = D // P  # 8 e-tiles

    with tile.TileContext(nc, pool_alloc_mode="queue") as tc, ExitStack() as top:
        cpool = top.enter_context(tc.tile_pool(name="const", bufs=1))
        ones_f = cpool.tile([P, 2], f32)
        nc.gpsimd.memset(ones_f, 32.0)  # folds the 32x of v~ into l
        ones = cpool.tile([P, 2], bf16)
        nc.vector.tensor_copy(ones, ones_f)

        # Persistent SBUF residents.
        qT_pool = top.enter_context(tc.tile_pool(name="qT", bufs=1))
        qT = [qT_pool.tile([P, S], bf16, name=f"qT{e}") for e in range(ET)]
        wkv_pool = top.enter_context(tc.tile_pool(name="wkv", bufs=1))
        wk_sb = wkv_pool.tile([P, ET, NS, P], fp8, name="wk")
        wv_sb = wkv_pool.tile([P, NS, D], fp8, name="wv")
        mpool = top.enter_context(tc.tile_pool(name="masks", bufs=1))
        masks_sb = mpool.tile([P, 8 * 512], bf16)
        xTp = top.enter_context(tc.tile_pool(name="xT", bufs=2))
        kv_pool = top.enter_context(tc.tile_pool(name="kv", bufs=2))
        ps_pool = top.enter_context(tc.tile_pool(name="ps", bufs=7, space="PSUM"))
        lp_pool = top.enter_context(tc.tile_pool(name="lp", bufs=1, space="PSUM"))

        # ---- DMA head: K0/V0 inputs first so PE starts early ----
        xT_tiles = [None] * NSTEP
        nc.sync.dma_start(wk_sb[:, 0:2], wk_d[:, 0 : 2 * NS * P])
        xT_tiles[0] = xTp.tile([P, NS, 256], fp8, tag="xT", name="xT0")
        nc.sync.dma_start(xT_tiles[0], xt_d[:, 0 : NS * 256])
        nc.sync.dma_start(wk_sb[:, 2:8], wk_d[:, 2 * NS * P : NS * D])
        nc.sync.dma_start(wv_sb, wv_d[:, :])

        def k_proj(xT_i):
            kTs = []
            for et in range(ET):
                pk = ps_pool.tile([P, 512], f32, tag="ps", name="pk")
                for j in range(0, NS, 2):
                    nc.tensor.matmul(
                        pk[:, 0:256],
                        lhsT=wk_sb[:, et, j : j + 2, :],
                        rhs=xT_i[:, j : j + 2, :],
                        start=(j == 0),
                        stop=(j == NS - 2),
                        perf_mode=DR,
                    )
                kt_sb = kv_pool.tile([P, 256], bf16, tag=f"kT{et}", name=f"kT{et}")
                nc.scalar.copy(kt_sb, pk[:, 0:256])
                kTs.append(kt_sb)
            return kTs

        def v_proj(xT_i):
            vs = []
            for st in range(2):
                v_sb = kv_pool.tile([P, D], bf16, tag=f"v{st}", name=f"v{st}")
                for eh in range(2):
                    pv = ps_pool.tile([P, 512], f32, tag="ps", name="pv")
                    for j in range(0, NS, 2):
                        nc.tensor.matmul(
                            pv,
                            lhsT=xT_i[:, j : j + 2, st * P : (st + 1) * P],
                            rhs=wv_sb[:, j : j + 2, eh * 512 : (eh + 1) * 512],
                            start=(j == 0),
                            stop=(j == NS - 2),
                            perf_mode=DR,
                        )
                    nc.scalar.copy(v_sb[:, eh * 512 : (eh + 1) * 512], pv)
                vs.append(v_sb)
            return vs

        # Step 0's K/V projection (runs while xq/wq stream in).
        kv0 = (k_proj(xT_tiles[0]), v_proj(xT_tiles[0]))

        # ---------------- Phase Q: qT projection (all 2048 queries) ----------------
        with ExitStack() as pq_scope:
            wq_pool = pq_scope.enter_context(tc.tile_pool(name="wq", bufs=1))
            wq_sb = wq_pool.tile([P, ET, NS, P], fp8)
            xq_pool = pq_scope.enter_context(tc.tile_pool(name="xq", bufs=2))

            nc.sync.dma_start(wq_sb, wq_d[:, :])
            nc.sync.dma_start(masks_sb, masks_d[:, :])
            for qh in range(2):
                xq_sb = xq_pool.tile([P, NS, 1024], fp8, tag="xq", name=f"xq{qh}")
                nc.sync.dma_start(
                    xq_sb, xq_d[:, qh * NS * 1024 : (qh + 1) * NS * 1024]
                )
                for et in range(ET):
                    for sb in range(2):
                        pq = ps_pool.tile([P, 512], f32, tag="ps", name="pq")
                        for j in range(0, NS, 2):
                            nc.tensor.matmul(
                                pq,
                                lhsT=wq_sb[:, et, j : j + 2, :],
                                rhs=xq_sb[:, j : j + 2, sb * 512 : (sb + 1) * 512],
                                start=(j == 0),
                                stop=(j == NS - 2),
                                perf_mode=DR,
                            )
                        nc.scalar.copy(
                            qT[et][:, qh * 1024 + sb * 512 : qh * 1024 + (sb + 1) * 512],
                            pq,
                        )

        # ---------------- Main loop: 4 own key blocks ----------------
        with ExitStack() as mn:
            acc_pool = mn.enter_context(tc.tile_pool(name="acc", bufs=1))
            O_sb = [acc_pool.tile([P, D], f32, name=f"O{j}") for j in range(16)]
            l_sb = acc_pool.tile([P, 16], f32)
            wt_pool = mn.enter_context(tc.tile_pool(name="wt", bufs=3))

            for i in range(NSTEP):
                W_i = S - 512 * i  # padded active width (cols 512*i..2048)
                n_i = W_i // P  # active slots
                c_base = 512 * i
                if i == 0:
                    kTs, vs = kv0
                else:
                    kTs, vs = k_proj(xT_tiles[i]), v_proj(xT_tiles[i])
                # Prefetch next step's xT.
                if i + 1 < NSTEP:
                    xT_tiles[i + 1] = xTp.tile(
                        [P, NS, 256], fp8, tag="xT", name=f"xT{i + 1}"
                    )
                    nc.sync.dma_start(
                        xT_tiles[i + 1],
                        xt_d[:, (i + 1) * NS * 256 : (i + 2) * NS * 256],
                    )

                # scoresT + exp (w in bf16, local cols 0..W_i).
                wts = []
                for kt in range(2):
                    wt = wt_pool.tile([P, S], bf16, tag="wt", name="wt")
                    for c0 in range(0, W_i, 512):
                        sp = ps_pool.tile([P, 512], f32, tag="ps", name="sp")
                        for et in range(ET):
                            nc.tensor.matmul(
                                sp,
                                lhsT=kTs[et][:, kt * P : (kt + 1) * P],
                                rhs=qT[et][:, c_base + c0 : c_base + c0 + 512],
                                start=(et == 0),
                                stop=(et == ET - 1),
                            )
                        # q~ k~ = 1024 qk -> fold /1024 into the exp scale.
                        nc.scalar.activation(
                            wt[:, c0 : c0 + 512],
                            sp,
                            mybir.ActivationFunctionType.Exp,
                            scale=float(SCALE / 1024.0),
                        )
                    wts.append(wt)
                # Causal masks on the first 4 slots (zero/tri/full per parity).
                for kt in range(2):
                    nc.vector.tensor_mul(
                        wts[kt][:, 0:512],
                        wts[kt][:, 0:512],
                        masks_sb[:, (2 * i + kt) * 512 : (2 * i + kt + 1) * 512],
                    )

                # O_j += w^T v~, l_j += w^T 32; slots 4i..4i+3 finish here.
                for jj in range(n_i - 1, -1, -1):
                    j = 4 * i + jj
                    for eh in range(2):
                        op = ps_pool.tile([P, 512], f32, tag="ps", name="op")
                        for kt in range(2):
                            nc.tensor.matmul(
                                op,
                                lhsT=wts[kt][:, jj * P : (jj + 1) * P],
                                rhs=vs[kt][:, eh * 512 : (eh + 1) * 512],
                                start=(kt == 0),
                                stop=(kt == 1),
                            )
                        dst = O_sb[j][:, eh * 512 : (eh + 1) * 512]
                        if i == 0:
                            nc.vector.tensor_copy(dst, op)
                        else:
                            nc.vector.tensor_add(dst, dst, op)
                        if jj < 4:
                            # Slot finished: ship unnormalized partial now.
                            nc.sync.dma_start(
                                out_d[j * P : (j + 1) * P, eh * 512 : (eh + 1) * 512],
                                dst,
                            )
                    lp = lp_pool.tile([P, 2], f32, tag="lp", name="lp")
                    for kt in range(2):
                        nc.tensor.matmul(
                            lp,
                            lhsT=wts[kt][:, jj * P : (jj + 1) * P],
                            rhs=ones,
                            start=(kt == 0),
                            stop=(kt == 1),
                        )
                    lcol = l_sb[:, j : j + 1]
                    if i == 0:
                        nc.vector.tensor_copy(lcol, lp[:, 0:1])
                    else:
                        nc.vector.tensor_add(lcol, lcol, lp[:, 0:1])
            nc.sync.dma_start(l_d[:, :], l_sb)

    nc.compile()
    return nc


def _fp8_variants(a):
    """a: [K, N] f32 -> (hi, lo16, hi16) f32 arrays (fp8-rounded values)."""
    import ml_dtypes

    f8 = ml_dtypes.float8_e4m3fn

    def q8(v):
        return v.astype(f8).astype(np.float32)

    hi = q8(a)
    lo16 = q8(16.0 * (a - hi))
    hi16 = q8(hi / 16.0)
    return hi, lo16, hi16


def _interleave(v0, v1, v2, ncols):
    """[K, ncols] variants -> [P, NS, ncols] fp8 in matmul slice order."""
    import ml_dtypes

    f8 = ml_dtypes.float8_e4m3fn
    KT = v0.shape[0] // P
    out = np.empty((P, 3 * KT, ncols), np.float32)
    for t in range(KT):
        sl = slice(t * P, (t + 1) * P)
        out[:, 3 * t + 0] = v0[sl]
        out[:, 3 * t + 1] = v1[sl]
        out[:, 3 * t + 2] = v2[sl]
    return np.ascontiguousarray(out.astype(f8))


def _host_inputs(x, Wq, Wk, Wv):
    import ml_dtypes

    bf = ml_dtypes.bfloat16

    # Slot pairing: x slots are (hi, lo16, hi16); W slots must be
    # (hi, hi16, lo16) so slot-wise products give the 3 cross terms.
    # W e-tile-major layout: [P, ET, NS, P].
    w_h = {}
    for name, w in (("Wq", Wq), ("Wk", Wk)):
        hi, lo16, hi16 = _fp8_variants(32.0 * w)
        il = _interleave(hi, hi16, lo16, D)  # [P, NS, D]
        w_h[name] = np.ascontiguousarray(
            il.reshape(P, NS, 8, P).transpose(0, 2, 1, 3).reshape(P, NS * D)
        )
    # Wv stays slice-major [P, NS, D] (moving operand, eh-sliced).
    hi, lo16, hi16 = _fp8_variants(32.0 * Wv)
    w_h["Wv"] = _interleave(hi, hi16, lo16, D).reshape(P, NS * D)

    in_maps = []
    for c in range(8):
        b, par = c // 2, c % 2
        own = OWN[par]
        xb = x[b]  # [S, D]
        xT = np.ascontiguousarray(xb.T)  # [D, S]
        hi, lo16, hi16 = _fp8_variants(xT)
        il = _interleave(hi, lo16, hi16, S)  # [P, NS, 2048]
        # xT: own key blocks, step-major
        xt_h = np.empty((P, NSTEP, NS, 256), np.float32)
        for i, g in enumerate(own):
            xt_h[:, i] = il[:, :, 256 * g : 256 * (g + 1)]
        xt_h = np.ascontiguousarray(
            xt_h.astype(il.dtype).reshape(P, NSTEP * NS * 256)
        )
        # xqT: all 2048 queries, qh-major halves
        xq_h = np.ascontiguousarray(
            il.reshape(P, NS, 2, 1024).transpose(0, 2, 1, 3).reshape(P, 2 * NS * 1024)
        )
        # masks: step i, kt, first 4 slots (cols = queries 512i..512i+512)
        masks = np.zeros((P, 8 * 512), np.float32)
        for i, g in enumerate(own):
            for kt in range(2):
                keys = 256 * g + 128 * kt + np.arange(P)[:, None]
                qs = 512 * i + np.arange(512)[None, :]
                masks[:, (2 * i + kt) * 512 : (2 * i + kt + 1) * 512] = (
                    keys <= qs
                ).astype(np.float32)
        in_maps.append(
            {
                "xT": xt_h,
                "xqT": xq_h,
                "Wq": w_h["Wq"],
                "Wk": w_h["Wk"],
                "Wv": w_h["Wv"],
                "masks": masks.astype(bf),
            }
        )
    return in_maps


def kernel(x, Wq, Wk, Wv):
    from concourse.bass_utils import run_bass_kernel_spmd

    x = np.asarray(x, dtype=np.float32)
    Wq = np.ascontiguousarray(np.asarray(Wq, dtype=np.float32))
    Wk = np.ascontiguousarray(np.asarray(Wk, dtype=np.float32))
    Wv = np.ascontiguousarray(np.asarray(Wv, dtype=np.float32))

    if "nc" not in _cached:
        _cached["nc"] = _build_bass()
    nc = _cached["nc"]

    in_maps = _host_inputs(x, Wq, Wk, Wv)
    res = run_bass_kernel_spmd(nc, in_maps, core_ids=list(range(8)))
    _cached["last_result"] = res

    out = np.zeros((B, S, D), np.float32)
    for b in range(B):
        rA, rB = res.results[2 * b], res.results[2 * b + 1]
        Osum = rA["out"].astype(np.float64) + rB["out"].astype(np.float64)
        lsum = (
            rA["l"].astype(np.float64) + rB["l"].astype(np.float64)
        ).T.reshape(S)
        out[b] = (Osum / lsum[:, None]).astype(np.float32)
    return out


# revision 32
# speedup vs baseline: 1.1625x; 1.1625x over previous
"""Single-head causal attention on 8 Trainium2 NeuronCores (Bass/Tile), v5.

v4 -> v5: key-split sharding. Each core pair splits the KEYS of its
batch item (core A takes 256-key blocks {0,3,4,7}, core B {1,2,5,6});
both cores process ALL 2048 queries and ship unnormalized partial
O~ = sum_own w v~ (f32) plus l = sum_own w; the host combines
(O~_A + O~_B) / (l_A + l_B). This dedupes the K/V projections across
the pair (the dominant PE cost in v3/v4) at the price of duplicating
the Q projection and ~7us of causal padding.

SPMD uniformity: both parities run the identical instruction stream --
step i processes one own key block at padded query width 2048-512*i;
the first 512 columns of each step are masked (zero/triangle/full
patterns shipped per core), which absorbs the parity difference.

Projections stay 3-term scaled-fp8 DoubleRow (see v4). Attention
tensors (qT, kT, v, w, masks) are bf16; O accumulates f32 in SBUF.
"""

import sys

import numpy as np

for _p in ("/opt/trn_rl_repo", "/root/.axon_site/_ro/trn_rl_repo"):
    if _p not in sys.path:
        sys.path.append(_p)

B, S, D = 4, 2048, 1024
P = 128
NS = 24  # fp8 variant slices per contraction (3 per 128-deep k-tile)
NSTEP = 4  # own key blocks per core (256 keys each)
OWN = {0: (0, 3, 4, 7), 1: (1, 2, 5, 6)}  # parity -> global key blocks
SCALE = 1.0 / np.sqrt(np.float32(D))

_cached = {}


def _build_bass():
    import concourse.bacc as bacc
    import concourse.mybir as mybir
    import concourse.tile as tile
    from contextlib import ExitStack

    f32 = mybir.dt.float32
    bf16 = mybir.dt.bfloat16
    fp8 = mybir.dt.float8e4
    DR = mybir.MatmulPerfMode.DoubleRow

    nc = bacc.Bacc("TRN2")
    # Host layouts (partition-major, contiguous rows):
    #   xT:    [128, i*(24*256) + s*256 + c]   own key blocks' x^T fp8 slices
    #   xqT:   [128, qh*(24*1024) + s*1024 + c] all queries' x^T fp8 slices
    #   W*:    [128, et*(24*128) + s*128 + e]   fp8 slices, e-tile-major
    #   masks: [128, (2*i+kt)*512 + c]          step i's first-4-slot masks
    xt_d = nc.declare_dram_parameter("xT", [P, NSTEP * NS * 256], fp8, isOutput=False)
    xq_d = nc.declare_dram_parameter("xqT", [P, 2 * NS * 1024], fp8, isOutput=False)
    wq_d = nc.declare_dram_parameter("Wq", [P, NS * D], fp8, isOutput=False)
    wk_d = nc.declare_dram_parameter("Wk", [P, NS * D], fp8, isOutput=False)
    wv_d = nc.declare_dram_parameter("Wv", [P, NS * D], fp8, isOutput=False)
    masks_d = nc.declare_dram_parameter("masks", [P, 8 * 512], bf16, isOutput=False)
    out_d = nc.declare_dram_parameter("out", [S, D], f32, isOutput=True)
    l_d = nc.declare_dram_parameter("l", [P, 16], f32, isOutput=True)

    ET = D // P  # 8 e-tiles

    with tile.TileContext(nc, pool_alloc_mode="queue") as tc, ExitStack() as top:
        cpool = top.enter_context(tc.tile_pool(name="const", bufs=1))
        ones_f = cpool.tile([P, 4], f32)
        nc.gpsimd.memset(ones_f, 32.0)  # folds the 32x of v~ into l
        ones8 = cpool.tile([P, 2, 2], fp8)
        nc.vector.tensor_copy(ones8, ones_f)

        # Persistent SBUF residents. qT holds fp8 hi (slices 0..7 per
        # e-tile) and lo = q~ - hi (slices 8..15): q~ is 32x-scaled so the
        # residual sits in e4m3's normal range, no extra scaling needed.
        qT_pool = top.enter_context(tc.tile_pool(name="qT", bufs=1))
        qT_all = qT_pool.tile([P, 2 * ET, S], fp8, name="qT")
        wkv_pool = top.enter_context(tc.tile_pool(name="wkv", bufs=1))
        wk_sb = wkv_pool.tile([P, ET, NS, P], fp8, name="wk")
        wv_sb = wkv_pool.tile([P, 2, NS, 512], fp8, name="wv")
        mpool = top.enter_context(tc.tile_pool(name="masks", bufs=1))
        masks_sb = mpool.tile([P, 8 * 512], bf16)
        xTp = top.enter_context(tc.tile_pool(name="xT", bufs=2))
        kv_pool = top.enter_context(tc.tile_pool(name="kv", bufs=2))
        ps_pool = top.enter_context(tc.tile_pool(name="ps", bufs=7, space="PSUM"))
        lp_pool = top.enter_context(tc.tile_pool(name="lp", bufs=1, space="PSUM"))

        xT_tiles = [None] * NSTEP

        def k_proj(xT_i):
            kTs = []
            for et in range(ET):
                pk = ps_pool.tile([P, 512], f32, tag="ps", name="pk")
                for j in range(0, NS, 2):
                    nc.tensor.matmul(
                        pk[:, 0:256],
                        lhsT=wk_sb[:, et, j : j + 2, :],
                        rhs=xT_i[:, j : j + 2, :],
                        start=(j == 0),
                        stop=(j == NS - 2),
                        perf_mode=DR,
                    )
                kt_sb = kv_pool.tile([P, 256], bf16, tag=f"kT{et}", name=f"kT{et}")
                nc.scalar.copy(kt_sb, pk[:, 0:256])
                kTs.append(kt_sb)
            return kTs

        def v_proj(xT_i):
            vs = []
            for st in range(2):
                v_sb = kv_pool.tile([P, D], bf16, tag=f"v{st}", name=f"v{st}")
                for eh in range(2):
                    pv = ps_pool.tile([P, 512], f32, tag="ps", name="pv")
                    for j in range(0, NS, 2):
                        nc.tensor.matmul(
                            pv,
                            lhsT=xT_i[:, j : j + 2, st * P : (st + 1) * P],
                            rhs=wv_sb[:, eh, j : j + 2, :],
                            start=(j == 0),
                            stop=(j == NS - 2),
                            perf_mode=DR,
                        )
                    nc.scalar.copy(v_sb[:, eh * 512 : (eh + 1) * 512], pv)
                vs.append(v_sb)
            return vs

        # ---------------- Phase Q first: qT projection (all 2048 queries) ----
        # DMA issue order = data-need order so PE starts ~6us in and never
        # starves: wq et-slabs + xq sb-slabs, then wk/xT0/wv for K0/V0.
        with ExitStack() as pq_scope:
            wq_pool = pq_scope.enter_context(tc.tile_pool(name="wq", bufs=1))
            wq_sb = wq_pool.tile([P, ET, NS, P], fp8)
            xq_pool = pq_scope.enter_context(tc.tile_pool(name="xq", bufs=1))
            # Query column slabs: 256,256 then 512x3 (small first chunks so
            # the very first matmul group starts ~4us in).
            QSLABS = (256, 256, 512, 512, 512)
            xq_sb = [
                xq_pool.tile([P, NS, w], fp8, tag=f"xq{si}", name=f"xq{si}")
                for si, w in enumerate(QSLABS)
            ]
            qoff = [0, 256, 512, 1024, 1536]

            SLAB = NS * P  # 3072 cols per wq e-tile slab
            A = 2 * (NS // 3)  # 16 slices: the self-sufficient 2/3 prefix
            nc.sync.dma_start(wq_sb[:, 0:1, 0:A, :], wq_d[:, 0 : A * P])
            nc.sync.dma_start(xq_sb[0][:, 0:A, :], xq_d[:, 0 : A * 256])
            nc.sync.dma_start(wq_sb[:, 0:1, A:NS, :], wq_d[:, A * P : SLAB])
            nc.sync.dma_start(xq_sb[0][:, A:NS, :], xq_d[:, A * 256 : NS * 256])
            for et in range(1, ET):
                nc.sync.dma_start(
                    wq_sb[:, et : et + 1], wq_d[:, et * SLAB : (et + 1) * SLAB]
                )
            off1 = NS * 256
            nc.sync.dma_start(xq_sb[1][:, 0:A, :], xq_d[:, off1 : off1 + A * 256])
            nc.sync.dma_start(xq_sb[1][:, A:NS, :], xq_d[:, off1 + A * 256 : 2 * off1])
            nc.sync.dma_start(xq_sb[2], xq_d[:, NS * 512 : 2 * NS * 512])
            nc.sync.dma_start(xq_sb[3], xq_d[:, 2 * NS * 512 : 3 * NS * 512])
            nc.sync.dma_start(xq_sb[4], xq_d[:, 3 * NS * 512 : 4 * NS * 512])
            nc.sync.dma_start(wk_sb, wk_d[:, :])
            xT_tiles[0] = xTp.tile([P, NS, 256], fp8, tag="xT", name="xT0")
            nc.sync.dma_start(xT_tiles[0], xt_d[:, 0 : NS * 256])
            nc.sync.dma_start(wv_sb[:, 0], wv_d[:, 0 : NS * 512])
            nc.sync.dma_start(wv_sb[:, 1], wv_d[:, NS * 512 : 2 * NS * 512])
            nc.sync.dma_start(masks_sb, masks_d[:, :])

            for si, w in enumerate(QSLABS):
                for et in range(ET):
                    pq = ps_pool.tile([P, 512], f32, tag="ps", name="pq")
                    for j in range(0, NS, 2):
                        nc.tensor.matmul(
                            pq[:, 0:w],
                            lhsT=wq_sb[:, et, j : j + 2, :],
                            rhs=xq_sb[si][:, j : j + 2, :],
                            start=(j == 0),
                            stop=(j == NS - 2),
                            perf_mode=DR,
                        )
                    nc.scalar.copy(qT[et][:, qoff[si] : qoff[si] + w], pq[:, 0:w])

        # Step 0's K/V projection.
        kv0 = (k_proj(xT_tiles[0]), v_proj(xT_tiles[0]))

        # ---------------- Main loop: 4 own key blocks ----------------
        with ExitStack() as mn:
            acc_pool = mn.enter_context(tc.tile_pool(name="acc", bufs=1))
            O_sb = [acc_pool.tile([P, D], f32, name=f"O{j}") for j in range(16)]
            l_sb = acc_pool.tile([P, 16], f32)
            wt_pool = mn.enter_context(tc.tile_pool(name="wt", bufs=3))

            for i in range(NSTEP):
                W_i = S - 512 * i  # padded active width (cols 512*i..2048)
                n_i = W_i // P  # active slots
                c_base = 512 * i
                if i == 0:
                    kTs, vs = kv0
                else:
                    kTs, vs = k_proj(xT_tiles[i]), v_proj(xT_tiles[i])
                # Prefetch next step's xT.
                if i + 1 < NSTEP:
                    xT_tiles[i + 1] = xTp.tile(
                        [P, NS, 256], fp8, tag="xT", name=f"xT{i + 1}"
                    )
                    nc.sync.dma_start(
                        xT_tiles[i + 1],
                        xt_d[:, (i + 1) * NS * 256 : (i + 2) * NS * 256],
                    )

                # scoresT + exp (w in bf16, local cols 0..W_i).
                wts = []
                for kt in range(2):
                    wt = wt_pool.tile([P, S], bf16, tag="wt", name="wt")
                    for c0 in range(0, W_i, 512):
                        sp = ps_pool.tile([P, 512], f32, tag="ps", name="sp")
                        for et in range(ET):
                            nc.tensor.matmul(
                                sp,
                                lhsT=kTs[et][:, kt * P : (kt + 1) * P],
                                rhs=qT[et][:, c_base + c0 : c_base + c0 + 512],
                                start=(et == 0),
                                stop=(et == ET - 1),
                            )
                        # q~ k~ = 1024 qk -> fold /1024 into the exp scale.
                        nc.scalar.activation(
                            wt[:, c0 : c0 + 512],
                            sp,
                            mybir.ActivationFunctionType.Exp,
                            scale=float(SCALE / 1024.0),
                        )
                    wts.append(wt)
                # Causal masks on the first 4 slots (zero/tri/full per parity).
                for kt in range(2):
                    nc.vector.tensor_mul(
                        wts[kt][:, 0:512],
                        wts[kt][:, 0:512],
                        masks_sb[:, (2 * i + kt) * 512 : (2 * i + kt + 1) * 512],
                    )

                # O_j += w^T v~, l_j += w^T 32; slots 4i..4i+3 finish here.
                for jj in range(n_i - 1, -1, -1):
                    j = 4 * i + jj
                    for eh in range(2):
                        op = ps_pool.tile([P, 512], f32, tag="ps", name="op")
                        for kt in range(2):
                            nc.tensor.matmul(
                                op,
                                lhsT=wts[kt][:, jj * P : (jj + 1) * P],
                                rhs=vs[kt][:, eh * 512 : (eh + 1) * 512],
                                start=(kt == 0),
                                stop=(kt == 1),
                            )
                        dst = O_sb[j][:, eh * 512 : (eh + 1) * 512]
                        # Pool helps drain psum mid-kernel; DVE alone is
                        # fastest for the final step's latency-critical drain.
                        eng = nc.vector if (eh == 0 or i == NSTEP - 1) else nc.gpsimd
                        if i == 0:
                            eng.tensor_copy(dst, op)
                        else:
                            eng.tensor_add(dst, dst, op)
                        if jj < 4:
                            # Slot finished: ship unnormalized partial now.
                            nc.sync.dma_start(
                                out_d[j * P : (j + 1) * P, eh * 512 : (eh + 1) * 512],
                                dst,
                            )
                    lp = lp_pool.tile([P, 2], f32, tag="lp", name="lp")
                    for kt in range(2):
                        nc.tensor.matmul(
                            lp,
                            lhsT=wts[kt][:, jj * P : (jj + 1) * P],
                            rhs=ones,
                            start=(kt == 0),
                            stop=(kt == 1),
                        )
                    lcol = l_sb[:, j : j + 1]
                    if i == 0:
                        nc.vector.tensor_copy(lcol, lp[:, 0:1])
                    else:
                        nc.vector.tensor_add(lcol, lcol, lp[:, 0:1])
                # Slots 4i..4i+3 are final: ship their l columns now.
                nc.sync.dma_start(l_d[:, 4 * i : 4 * i + 4], l_sb[:, 4 * i : 4 * i + 4])

    nc.compile()
    return nc


def _fp8_variants(a):
    """a: [K, N] f32 -> (hi, lo16, hi16) f32 arrays (fp8-rounded values)."""
    import ml_dtypes

    f8 = ml_dtypes.float8_e4m3fn

    def q8(v):
        return v.astype(f8).astype(np.float32)

    hi = q8(a)
    lo16 = q8(16.0 * (a - hi))
    hi16 = q8(hi / 16.0)
    return hi, lo16, hi16


def _interleave(v0, v1, v2, ncols):
    """[K, ncols] variants -> [P, NS, ncols] fp8 in matmul slice order.

    Slices 0..15 pair (v0_t, v1_t) per k-tile t; slices 16..23 are v2_t.
    With x=(hi, l16, h16) and W=(hi, h16, l16) the uniform j:j+2 DoubleRow
    stepping yields x_hi*W_hi + x_l16*W_h16 (j<16) then x_h16*W_l16
    (j>=16) -- and the first 2/3 of each tensor is usable on its own,
    which lets the head DMA split at the 2/3 mark.
    """
    import ml_dtypes

    f8 = ml_dtypes.float8_e4m3fn
    KT = v0.shape[0] // P
    out = np.empty((P, 3 * KT, ncols), np.float32)
    for t in range(KT):
        sl = slice(t * P, (t + 1) * P)
        out[:, 2 * t + 0] = v0[sl]
        out[:, 2 * t + 1] = v1[sl]
        out[:, 2 * KT + t] = v2[sl]
    return np.ascontiguousarray(out.astype(f8))


def _host_inputs(x, Wq, Wk, Wv):
    import ml_dtypes

    bf = ml_dtypes.bfloat16

    # Slot pairing: x slots are (hi, lo16, hi16); W slots must be
    # (hi, hi16, lo16) so slot-wise products give the 3 cross terms.
    # W e-tile-major layout: [P, ET, NS, P].
    w_h = {}
    for name, w in (("Wq", Wq), ("Wk", Wk)):
        hi, lo16, hi16 = _fp8_variants(32.0 * w)
        il = _interleave(hi, hi16, lo16, D)  # [P, NS, D]
        w_h[name] = np.ascontiguousarray(
            il.reshape(P, NS, 8, P).transpose(0, 2, 1, 3).reshape(P, NS * D)
        )
    # Wv is the moving operand, eh-major: [P, 2, NS, 512].
    hi, lo16, hi16 = _fp8_variants(32.0 * Wv)
    il = _interleave(hi, hi16, lo16, D)
    w_h["Wv"] = np.ascontiguousarray(
        il.reshape(P, NS, 2, 512).transpose(0, 2, 1, 3).reshape(P, NS * D)
    )

    in_maps = []
    for c in range(8):
        b, par = c // 2, c % 2
        own = OWN[par]
        xb = x[b]  # [S, D]
        xT = np.ascontiguousarray(xb.T)  # [D, S]
        hi, lo16, hi16 = _fp8_variants(xT)
        il = _interleave(hi, lo16, hi16, S)  # [P, NS, 2048]
        # xT: own key blocks, step-major
        xt_h = np.empty((P, NSTEP, NS, 256), np.float32)
        for i, g in enumerate(own):
            xt_h[:, i] = il[:, :, 256 * g : 256 * (g + 1)]
        xt_h = np.ascontiguousarray(
            xt_h.astype(il.dtype).reshape(P, NSTEP * NS * 256)
        )
        # xqT: all 2048 queries, slab-major (256,256,512,512,512 cols)
        xq_h = np.ascontiguousarray(
            np.concatenate(
                [
                    il[:, :, c0 : c0 + w].reshape(P, NS * w)
                    for c0, w in ((0, 256), (256, 256), (512, 512), (1024, 512), (1536, 512))
                ],
                axis=1,
            )
        )
        # masks: step i, kt, first 4 slots (cols = queries 512i..512i+512)
        masks = np.zeros((P, 8 * 512), np.float32)
        for i, g in enumerate(own):
            for kt in range(2):
                keys = 256 * g + 128 * kt + np.arange(P)[:, None]
                qs = 512 * i + np.arange(512)[None, :]
                masks[:, (2 * i + kt) * 512 : (2 * i + kt + 1) * 512] = (
                    keys <= qs
                ).astype(np.float32)
        in_maps.append(
            {
                "xT": xt_h,
                "xqT": xq_h,
                "Wq": w_h["Wq"],
                "Wk": w_h["Wk"],
                "Wv": w_h["Wv"],
                "masks": masks.astype(bf),
            }
        )
    return in_maps


def kernel(x, Wq, Wk, Wv):
    from concourse.bass_utils import run_bass_kernel_spmd

    x = np.asarray(x, dtype=np.float32)
    Wq = np.ascontiguousarray(np.asarray(Wq, dtype=np.float32))
    Wk = np.ascontiguousarray(np.asarray(Wk, dtype=np.float32))
    Wv = np.ascontiguousarray(np.asarray(Wv, dtype=np.float32))

    if "nc" not in _cached:
        _cached["nc"] = _build_bass()
    nc = _cached["nc"]

    in_maps = _host_inputs(x, Wq, Wk, Wv)
    res = run_bass_kernel_spmd(nc, in_maps, core_ids=list(range(8)))
    _cached["last_result"] = res

    out = np.zeros((B, S, D), np.float32)
    for b in range(B):
        rA, rB = res.results[2 * b], res.results[2 * b + 1]
        Osum = rA["out"].astype(np.float64) + rB["out"].astype(np.float64)
        lsum = (
            rA["l"].astype(np.float64) + rB["l"].astype(np.float64)
        ).T.reshape(S)
        out[b] = (Osum / lsum[:, None]).astype(np.float32)
    return out
